# revision 1
# baseline (speedup 1.0000x reference)
"""Contrastive loss (SimCLR-style) TRN2 Bass kernel, 8-core data-parallel.

Math: z [8192, 256] f32 ->
  zn = z / ||z||row ; S = (zn @ zn.T)/0.1 ; diag masked; row log_softmax;
  loss = -mean_i( S[i, pos(i)] - logsumexp_j S[i, j] ), pos(i) = (i+4096) % 8192.

Strategy (per sharding hint): shard rows across 8 cores (1024 rows each).
Each core receives the full z (replicated), its own row block, and the
partner row block. On device: cast to bf16, normalize, build the transposed
normalized matrix via X-bar DMA transpose through a DRAM bounce, then for
each of 8 m-tiles x 4 col-groups: bf16 matmul into a 4-bank PSUM group and
a single fused ACT Exp(10x-10) + row-accumulate. The fixed shift of -10 is
safe because logits/T lie in [-10, 10]. The self term exp(10*d_ii - 10) is
subtracted analytically from the row sum (d_ii computed on-chip), and the
positive logit d_pos comes from an elementwise dot with the partner block,
keeping the hot loop fully uniform. Device outputs per-row d_pos and
ln(rowsum); the host gather computes loss = -mean(10*d_pos - 10 - ln_rs).
"""

import numpy as np

N = 8192
K = 256
N_CORES = 8
BLK = N // N_CORES          # 1024 rows per core
MT = BLK // 128             # 8 m-tiles per core
NT = N // 128               # 64 row tiles of full z
NGROUP = 4                  # col groups per m-tile (2048 cols each)
GW = N // NGROUP            # 2048 cols per group
TEMP_INV = 10.0             # 1/temperature

_CACHE = {}


def _build():
    import concourse.bass as bass
    import concourse.tile as tile
    from concourse import bacc, mybir
    from concourse.bass_interp import get_hw_module

    F32, BF16 = mybir.dt.float32, mybir.dt.bfloat16
    AF, ALU = mybir.ActivationFunctionType, mybir.AluOpType
    AX = mybir.AxisListType

    nc = bacc.Bacc("TRN2", target_bir_lowering=False, debug=False,
                   enable_asserts=False, num_devices=N_CORES)

    zf_in = nc.dram_tensor("zf", [N, K], F32, kind="ExternalInput").ap()
    zb_in = nc.dram_tensor("zb", [BLK, K], F32, kind="ExternalInput").ap()
    zp_in = nc.dram_tensor("zp", [BLK, K], F32, kind="ExternalInput").ap()
    dpos_out = nc.dram_tensor("dpos", [128, MT], F32, kind="ExternalOutput").ap()
    lnrs_out = nc.dram_tensor("lnrs", [128, MT], F32, kind="ExternalOutput").ap()

    with tile.TileContext(nc) as tc:
        with (
            tc.tile_pool(name="big", bufs=1) as big,
            tc.tile_pool(name="work", bufs=4) as work,
            tc.tile_pool(name="stat", bufs=1) as stat,
            tc.tile_pool(name="dram", bufs=1, space=bass.MemorySpace.DRAM) as dram,
            tc.tile_pool(name="ps", bufs=2, space=bass.MemorySpace.PSUM) as psp,
        ):
            # ---- load with f32->bf16 cast (SWDGE) ----
            zbf = big.tile([128, NT, K], BF16)
            nc.gpsimd.dma_start(zbf[:], zf_in.rearrange("(t p) k -> p t k", p=128))
            zbf_b = big.tile([128, MT, K], BF16)
            nc.gpsimd.dma_start(zbf_b[:], zb_in.rearrange("(t p) k -> p t k", p=128))
            zbf_p = big.tile([128, MT, K], BF16)
            nc.gpsimd.dma_start(zbf_p[:], zp_in.rearrange("(t p) k -> p t k", p=128))

            # ---- row norms: ss = sum(z^2); rsq = exp(-0.5*ln(ss)) ----
            def rownorm(src, nt, tag):
                ss = stat.tile([128, nt], F32, tag=f"ss_{tag}")
                for t in range(nt):
                    sq = work.tile([128, K], F32, tag="sq")
                    nc.vector.tensor_mul(sq[:], src[:, t, :], src[:, t, :])
                    nc.vector.reduce_sum(ss[:, t:t + 1], sq[:], axis=AX.X)
                lnss = stat.tile([128, nt], F32, tag=f"ln_{tag}")
                nc.scalar.activation(lnss[:], ss[:], AF.Ln)
                rsq = stat.tile([128, nt], F32, tag=f"rs_{tag}")
                nc.scalar.activation(rsq[:], lnss[:], AF.Exp, scale=-0.5)
                return rsq

            rsq_f = rownorm(zbf, NT, "f")
            rsq_b = rownorm(zbf_b, MT, "b")
            rsq_p = rownorm(zbf_p, MT, "p")

            # ---- normalize -> bf16 ----
            zn = big.tile([128, NT, K], BF16)
            for t in range(NT):
                nc.vector.tensor_scalar(zn[:, t, :], zbf[:, t, :],
                                        rsq_f[:, t:t + 1], None, op0=ALU.mult)
            zn_b = big.tile([128, MT, K], BF16)
            zn_p = big.tile([128, MT, K], BF16)
            for t in range(MT):
                nc.vector.tensor_scalar(zn_b[:, t, :], zbf_b[:, t, :],
                                        rsq_b[:, t:t + 1], None, op0=ALU.mult)
                nc.vector.tensor_scalar(zn_p[:, t, :], zbf_p[:, t, :],
                                        rsq_p[:, t:t + 1], None, op0=ALU.mult)

            # ---- d_ii and d_pos (per-row dots, f32) ----
            d_ii = stat.tile([128, MT], F32)
            d_pos = stat.tile([128, MT], F32)
            for t in range(MT):
                sq = work.tile([128, K], F32, tag="sq")
                nc.vector.tensor_mul(sq[:], zn_b[:, t, :], zn_b[:, t, :])
                nc.vector.reduce_sum(d_ii[:, t:t + 1], sq[:], axis=AX.X)
                sq2 = work.tile([128, K], F32, tag="sq")
                nc.vector.tensor_mul(sq2[:], zn_b[:, t, :], zn_p[:, t, :])
                nc.vector.reduce_sum(d_pos[:, t:t + 1], sq2[:], axis=AX.X)

            # ---- bounce zn to DRAM and transpose-load -> znT [256, N] ----
            # split into NGROUP row groups so transpose loads pipeline behind stores
            TPG = NT // NGROUP  # row tiles per group
            znT0 = big.tile([128, N], BF16)
            znT1 = big.tile([128, N], BF16)
            for g in range(NGROUP):
                zng = dram.tile([TPG * 128, K], BF16, tag=f"zng{g}")
                nc.sync.dma_start(zng.rearrange("(t p) k -> p t k", p=128),
                                  zn[:, g * TPG:(g + 1) * TPG, :])
                sl = slice(g * GW, (g + 1) * GW)
                nc.sync.dma_start(znT0[:, sl], zng[:, 0:128], transpose=True)
                nc.sync.dma_start(znT1[:, sl], zng[:, 128:256], transpose=True)

            znb_dram = dram.tile([BLK, K], BF16)
            nc.sync.dma_start(znb_dram.rearrange("(t p) k -> p t k", p=128), zn_b[:])
            znTb0 = big.tile([128, BLK], BF16)
            znTb1 = big.tile([128, BLK], BF16)
            nc.sync.dma_start(znTb0[:], znb_dram[:, 0:128], transpose=True)
            nc.sync.dma_start(znTb1[:], znb_dram[:, 128:256], transpose=True)

            # ---- main loop: matmul groups + fused exp/accum ----
            bias_m10 = stat.tile([128, 1], F32)
            nc.vector.memset(bias_m10[:], -TEMP_INV)
            rs_part = stat.tile([128, MT, NGROUP], F32)
            for mt in range(MT):
                lhs0 = znTb0[:, mt * 128:(mt + 1) * 128]
                lhs1 = znTb1[:, mt * 128:(mt + 1) * 128]
                for g in range(NGROUP):
                    ps = psp.tile([128, GW], F32, tag="ps")
                    for sub in range(GW // 512):
                        psl = slice(sub * 512, (sub + 1) * 512)
                        nsl = slice(g * GW + sub * 512, g * GW + (sub + 1) * 512)
                        nc.tensor.matmul(ps[:, psl], lhs0, znT0[:, nsl],
                                         start=True, stop=False)
                        nc.tensor.matmul(ps[:, psl], lhs1, znT1[:, nsl],
                                         start=False, stop=True)
                    expo = work.tile([128, GW], BF16, tag="expo")
                    nc.scalar.activation(expo[:], ps[:], AF.Exp,
                                         bias=bias_m10[:], scale=TEMP_INV,
                                         accum_out=rs_part[:, mt, g:g + 1])

            # ---- epilogue ----
            rs_sum = stat.tile([128, MT], F32)
            nc.vector.reduce_sum(rs_sum[:], rs_part[:], axis=AX.X)
            self_t = stat.tile([128, MT], F32)
            nc.scalar.activation(self_t[:], d_ii[:], AF.Exp,
                                 bias=bias_m10[:], scale=TEMP_INV)
            rs_corr = stat.tile([128, MT], F32)
            nc.vector.tensor_sub(rs_corr[:], rs_sum[:], self_t[:])
            lnrs = stat.tile([128, MT], F32)
            nc.scalar.activation(lnrs[:], rs_corr[:], AF.Ln)

            nc.sync.dma_start(dpos_out, d_pos[:])
            nc.sync.dma_start(lnrs_out, lnrs[:])

    nc.compile()
    nc.m = get_hw_module(nc.m)
    return nc


def _get_nc():
    if "nc" not in _CACHE:
        _CACHE["nc"] = _build()
    return _CACHE["nc"]


def _in_maps(z):
    z = np.ascontiguousarray(z, dtype=np.float32)
    maps = []
    for c in range(N_CORES):
        r0 = c * BLK
        p0 = (r0 + N // 2) % N
        maps.append({
            "zf": z,
            "zb": np.ascontiguousarray(z[r0:r0 + BLK]),
            "zp": np.ascontiguousarray(z[p0:p0 + BLK]),
        })
    return maps


def _finish(results):
    total = 0.0
    for c in range(N_CORES):
        dpos = results[c]["dpos"].astype(np.float64)
        lnrs = results[c]["lnrs"].astype(np.float64)
        total += (TEMP_INV * dpos - TEMP_INV - lnrs).sum()
    return np.float32(-total / N)


def kernel(z):
    from concourse import bass_utils
    nc = _get_nc()
    res = bass_utils.run_bass_kernel_spmd(nc, _in_maps(z),
                                          core_ids=list(range(N_CORES)))
    return _finish(res.results)


# revision 12
# speedup vs baseline: 9906.3294x; 9906.3294x over previous
"""Contrastive loss (SimCLR-style) TRN2 Bass kernel, 8-core data-parallel.

Math: z [8192, 256] f32 ->
  zn = z / ||z||row ; S = (zn @ zn.T)/0.1 ; diag masked; row log_softmax;
  loss = -mean_i( S[i, pos(i)] - logsumexp_j S[i, j] ), pos(i) = (i+4096) % 8192.

Strategy (per sharding hint): shard rows across 8 cores (1024 rows each).
Each core receives the full z (replicated), its own row block, and the
partner row block. On device, in 4 pipelined column groups: cast to bf16 on
load, row-normalize (rsqrt = Quake bit-trick + 2 Newton steps, all on DVE so
the ACT engine keeps a single Exp table set), bounce to DRAM and X-bar
DMA-transpose back as znT [256, cols], then 8 m-tiles of bf16 matmul into
4-bank PSUM groups, each drained by one fused ACT Exp(10x-10) +
row-accumulate. The fixed -10 shift is safe since logits/T lie in [-10, 10].
The self term exp(10*d_ii-10) is subtracted analytically and the positive
logit comes from an elementwise dot with the partner block, so the hot loop
is fully uniform. Device outputs per-row d_pos and the corrected rowsum;
the host gather computes loss = -mean(10*d_pos - 10 - log(rowsum)).
"""

import numpy as np

N = 8192
K = 256
N_CORES = 8
BLK = N // N_CORES          # 1024 rows per core
MT = BLK // 128             # 8 m-tiles per core
NT = N // 128               # 64 row tiles of full z
GROUP_TILES = [4, 4, 8, 16, 16, 16]  # row tiles per pipeline group (sums to NT)
NGROUP = len(GROUP_TILES)
TEMP_INV = 10.0             # 1/temperature
QMAGIC = 0x5F3759DF

_CACHE = {}


def _build(amp=1):
    import concourse.bass as bass
    import concourse.tile as tile
    from concourse import bacc, mybir
    from concourse.bass_interp import get_hw_module

    F32, BF16 = mybir.dt.float32, mybir.dt.bfloat16
    I32 = mybir.dt.int32
    AF, ALU = mybir.ActivationFunctionType, mybir.AluOpType
    AX = mybir.AxisListType

    nc = bacc.Bacc("TRN2", target_bir_lowering=False, debug=False,
                   enable_asserts=False, num_devices=N_CORES)

    zf_in = nc.dram_tensor("zf", [N, K], F32, kind="ExternalInput").ap()
    zb_in = nc.dram_tensor("zb", [BLK, K], F32, kind="ExternalInput").ap()
    zp_in = nc.dram_tensor("zp", [BLK, K], F32, kind="ExternalInput").ap()
    dpos_out = nc.dram_tensor("dpos", [128, MT], F32, kind="ExternalOutput").ap()
    rs_out = nc.dram_tensor("rs", [128, MT], F32, kind="ExternalOutput").ap()

    with tile.TileContext(nc) as tc:
        with (
            tc.tile_pool(name="big", bufs=1) as big,
            tc.tile_pool(name="pipe", bufs=3) as pipe,
            tc.tile_pool(name="work", bufs=2) as work,
            tc.tile_pool(name="stat", bufs=1) as stat,
            tc.tile_pool(name="dram", bufs=1, space=bass.MemorySpace.DRAM) as dram,
            tc.tile_pool(name="ps", bufs=2, space=bass.MemorySpace.PSUM) as psp,
        ):
            magic = stat.tile([128, NT], I32)
            nc.vector.memset(magic[:], QMAGIC)

            def rsqrt_dve(ss, nt, tag):
                """rsq = 1/sqrt(ss), Quake init + 3 Newton iterations (DVE)."""
                ssi = ss[:].bitcast(I32)
                sh = work.tile([128, nt], I32, tag="sh")
                nc.vector.tensor_scalar(sh[:], ssi, 1, None,
                                        op0=ALU.arith_shift_right)
                y = stat.tile([128, nt], F32, tag=f"y_{tag}")
                yi = y[:].bitcast(I32)
                # magic - (i >> 1)
                nc.vector.tensor_sub(yi, magic[:, 0:nt], sh[:])
                for it in range(3):
                    y2 = work.tile([128, nt], F32, tag="nwt")
                    nc.vector.tensor_mul(y2[:], y[:], y[:])
                    xy2 = work.tile([128, nt], F32, tag="nwt")
                    nc.vector.tensor_mul(xy2[:], ss[:], y2[:])
                    c = work.tile([128, nt], F32, tag="nwt")
                    nc.vector.tensor_scalar(c[:], xy2[:], -0.5, 1.5,
                                            op0=ALU.mult, op1=ALU.add)
                    yn = stat.tile([128, nt], F32, tag=f"y{it}_{tag}")
                    nc.vector.tensor_mul(yn[:], y[:], c[:])
                    y = yn
                return y

            def norm_chain(zbf, nt, tag, znT0, znT1, col0, mul_engine=None):
                """normalize pre-loaded bf16 rows -> zn + transposed halves."""
                me = mul_engine or nc.vector
                ss = stat.tile([128, nt], F32, tag=f"ss_{tag}")
                for h0 in range(0, nt, 8):
                    h1 = min(h0 + 8, nt)
                    sq = work.tile([128, h1 - h0, K], F32, tag="sq")
                    me.tensor_mul(sq[:], zbf[:, h0:h1, :], zbf[:, h0:h1, :])
                    nc.vector.reduce_sum(ss[:, h0:h1], sq[:], axis=AX.X)
                rsq = rsqrt_dve(ss, nt, tag)
                zn = pipe.tile([128, nt, K], BF16, tag="zn")
                for t in range(nt):
                    nc.vector.tensor_scalar(zn[:, t, :], zbf[:, t, :],
                                            rsq[:, t:t + 1], None, op0=ALU.mult)
                zn_dram = dram.tile([nt * 128, K], BF16, tag=f"znd_{tag}")
                nc.sync.dma_start(zn_dram.rearrange("(t p) k -> p t k", p=128), zn[:])
                w = nt * 128
                nc.sync.dma_start(znT0[:, col0:col0 + w], zn_dram[:, 0:128],
                                  transpose=True)
                nc.sync.dma_start(znT1[:, col0:col0 + w], zn_dram[:, 128:256],
                                  transpose=True)
                return zn

            # ---- own block + partner block (feeds lhsT, d_ii, d_pos) ----
            znTb0 = big.tile([128, BLK], BF16)
            znTb1 = big.tile([128, BLK], BF16)
            zbf_b = big.tile([128, MT, K], BF16, tag="zbf_b")
            nc.gpsimd.dma_start(zbf_b[:], zb_in.rearrange("(t p) k -> p t k", p=128))
            g_zbf = []
            _c0 = 0
            for g, tpg in enumerate(GROUP_TILES):
                zt = pipe.tile([128, tpg, K], BF16, tag="zbf")
                nc.gpsimd.dma_start(
                    zt[:], zf_in[_c0:_c0 + tpg * 128, :].rearrange(
                        "(t p) k -> p t k", p=128))
                g_zbf.append(zt)
                _c0 += tpg * 128
            zn_b = norm_chain(zbf_b, MT, "b", znTb0, znTb1, 0)

            # ---- main pipeline over column groups (small first for fast start) ----
            bias_m10 = stat.tile([128, 1], F32)
            nc.vector.memset(bias_m10[:], -TEMP_INV)
            rs_part = stat.tile([128, MT, NGROUP], F32)
            col0 = 0
            for g, tpg in enumerate(GROUP_TILES):
                gw = tpg * 128
                znT0 = pipe.tile([128, gw], BF16, tag="znT0")
                znT1 = pipe.tile([128, gw], BF16, tag="znT1")
                norm_chain(g_zbf[g], tpg, f"g{g}", znT0, znT1, 0)
                col0 += gw
                for mt in range(MT * amp):
                    mt = mt % MT
                    lhs0 = znTb0[:, mt * 128:(mt + 1) * 128]
                    lhs1 = znTb1[:, mt * 128:(mt + 1) * 128]
                    ps = psp.tile([128, gw], F32, tag="ps")
                    for sub in range(gw // 512):
                        psl = slice(sub * 512, (sub + 1) * 512)
                        nc.tensor.matmul(ps[:, psl], lhs0, znT0[:, psl],
                                         start=True, stop=False)
                        nc.tensor.matmul(ps[:, psl], lhs1, znT1[:, psl],
                                         start=False, stop=True)
                    expo = work.tile([128, gw], BF16, tag="expo")
                    nc.scalar.activation(expo[:], ps[:], AF.Exp,
                                         bias=bias_m10[:], scale=TEMP_INV,
                                         accum_out=rs_part[:, mt, g:g + 1])

            zbf_p = big.tile([128, MT, K], BF16, tag="zbf_p")
            nc.gpsimd.dma_start(zbf_p[:], zp_in.rearrange("(t p) k -> p t k", p=128))
            sq_p = work.tile([128, MT, K], F32, tag="sq")
            nc.vector.tensor_mul(sq_p[:], zbf_p[:], zbf_p[:])
            ss_p = stat.tile([128, MT], F32)
            nc.vector.reduce_sum(ss_p[:], sq_p[:], axis=AX.X)
            rsq_p = rsqrt_dve(ss_p, MT, "p")
            zn_p = big.tile([128, MT, K], BF16)
            for t in range(MT):
                nc.vector.tensor_scalar(zn_p[:, t, :], zbf_p[:, t, :],
                                        rsq_p[:, t:t + 1], None, op0=ALU.mult)

            # ---- per-row self and positive dots (fill DVE gaps late) ----
            d_ii = stat.tile([128, MT], F32)
            d_pos = stat.tile([128, MT], F32)
            sqb = work.tile([128, MT, K], F32, tag="sq")
            nc.vector.tensor_mul(sqb[:], zn_b[:], zn_b[:])
            nc.vector.reduce_sum(d_ii[:], sqb[:], axis=AX.X)
            sqp = work.tile([128, MT, K], F32, tag="sq")
            nc.vector.tensor_mul(sqp[:], zn_b[:], zn_p[:])
            nc.vector.reduce_sum(d_pos[:], sqp[:], axis=AX.X)

            # ---- epilogue: rowsum minus self term ----
            rs_sum = stat.tile([128, MT], F32)
            nc.vector.reduce_sum(rs_sum[:], rs_part[:], axis=AX.X)
            self_t = stat.tile([128, MT], F32)
            nc.scalar.activation(self_t[:], d_ii[:], AF.Exp,
                                 bias=bias_m10[:], scale=TEMP_INV)
            rs_corr = stat.tile([128, MT], F32)
            nc.vector.tensor_sub(rs_corr[:], rs_sum[:], self_t[:])

            nc.sync.dma_start(dpos_out, d_pos[:])
            nc.sync.dma_start(rs_out, rs_corr[:])

    nc.compile()
    nc.m = get_hw_module(nc.m)
    return nc


def _get_nc(amp=1):
    key = f"nc{amp}"
    if key not in _CACHE:
        _CACHE[key] = _build(amp)
    return _CACHE[key]


def _in_maps(z):
    z = np.ascontiguousarray(z, dtype=np.float32)
    maps = []
    for c in range(N_CORES):
        r0 = c * BLK
        p0 = (r0 + N // 2) % N
        maps.append({
            "zf": z,
            "zb": np.ascontiguousarray(z[r0:r0 + BLK]),
            "zp": np.ascontiguousarray(z[p0:p0 + BLK]),
        })
    return maps


def _finish(results):
    total = 0.0
    for c in range(N_CORES):
        dpos = results[c]["dpos"].astype(np.float64)
        rs = results[c]["rs"].astype(np.float64)
        total += (TEMP_INV * dpos - TEMP_INV - np.log(rs)).sum()
    return np.float32(-total / N)


def kernel(z):
    from concourse import bass_utils
    nc = _get_nc()
    res = bass_utils.run_bass_kernel_spmd(nc, _in_maps(z),
                                          core_ids=list(range(N_CORES)))
    return _finish(res.results)
